# revision 25
# baseline (speedup 1.0000x reference)
"""Trainium2 Bass kernel for nn_DistAttn (GNN edge-softmax message passing).

Strategy (8 NeuronCores, SPMD single program), "design V":
  - Destination-node sharding: nodes packed into 320 bins (8 cores x 40
    blocks of <=128 dst slots) by a degree-balancing greedy; every edge
    lives on exactly one core; per-block edge counts near-uniform.
  - Score refactor: e = f_dst (Wq Wk^T) f_src^T, so K rows are the RAW
    bf16 features (host-supplied table featR, available at t=0) and the
    per-block query tiles are QT[c,d] = B^T @ feat_blk^T with the tiny
    host-computed B = Wq @ Wk^T.  Only the h table (feat @ W_fc) is
    built on device (phase 1), as bf16 DRAM rows split low/high at
    row 22528 so gather indices fit int16 and the low half completes
    early.
  - Phase 1 (per core): Htab = feat @ W_fc for all 40064 padded nodes
    (PE), PSUM evacuated by a cost-balanced mix of Pool/ACT/DVE copies,
    streamed to DRAM on SP/ACT/Pool queues.  QT tiles computed first and
    kept in SBUF.  K^T gathers for the first groups run on Pool from
    t~0 (featR needs no compute).
  - Phase 2: edges in groups of GPB=2 blocks.  Per group, Pool issues
    transposed gathers of K^T [c,j] from featR (elem 256B) and int64-
    viewed row gathers of h (32 elems/idx) from Htab, sharing one int16
    index array.  Per 128-edge tile, PE computes all-pairs scores
    S^T[j,d] = K^T.T @ QT into PSUM, then adds 256*onehot(dst) with a
    single fp8 DoubleRow matmul (lhsT = host-built one-hot in [64,2,128]
    layout, rhs = 256*I in the matching layout; 0.5 cycles/row).  One
    ACT Exp over an 8-tile PSUM region yields the masked softmax
    numerators M2[j,d] = exp(e_j - c0) * onehot (mismatches underflow:
    exp(-256/sqrt(128)) ~ 1.5e-10).  Two more PE matmuls accumulate
    U[d,:] += M2^T @ h and denom[d] += M2^T @ 1.  Block epilogue (DVE):
    reciprocal + scaled copy, then DMA the rows out on SP.
  - exp shift c0 >= max e is host-computed, so no segment-max pass.
  - The host unpermutes the output rows and zeroes deg-0 nodes.
"""

import sys

sys.path.insert(0, "/opt/trn_rl_repo")

import numpy as np

import concourse.bacc as bacc
import concourse.mybir as mybir
import concourse.tile as tile
from concourse.bass_utils import run_bass_kernel_spmd
from concourse.library_config import mlp as mlp_lib

dt = mybir.dt
BF16 = dt.np(dt.bfloat16)
FP8 = dt.np(dt.float8e4)

N = 40000
E = 640000
F = 128
CORES = 8
NPC = N // CORES            # 5000 dst nodes per core
BLK = 128                   # dst nodes per block
NBLK = (NPC + BLK - 1) // BLK   # 40 blocks per core
SPLIT = 22528               # low/high table split; both halves < 2**15 rows
NPADT = 40064               # node count padded to 128 multiple (313 tiles)
NT_GLOBAL = NPADT // 128    # 313
SCALE = float(np.sqrt(np.float32(F)))
BIGSCALE = 224.0            # exactly representable in fp8e4 (max 240)
GPB = 2                     # blocks per gather group
NG = NBLK // GPB            # groups per core
OCT = 8                     # tiles per ACT exp call
PFK = 5                     # K-gather groups prefetched from t=0
USE_DR = True               # fp8 DoubleRow mask matmul (0.5 cycles/row)
USE_I64 = False             # int64 gathers unsupported by the HW backend
DEFER = 3                   # S/exp groups run ahead of their U groups


def _pack_nodes(deg_low, deg_high):
    """Assign nodes to CORES*NBLK bins (<=128 nodes each), balancing the
    per-bin low/high edge counts to minimize gather padding."""
    import heapq
    nbins = CORES * NBLK
    nodes = np.argsort(-(deg_low + deg_high), kind="stable")
    # caps target whole tile counts: ceil(avg/128) tiles per (bin, half)
    cap_l = max(np.ceil(float(deg_low.sum()) / nbins / 128) * 128 - 2.0, 1.0)
    cap_h = max(np.ceil(float(deg_high.sum()) / nbins / 128) * 128 - 2.0, 1.0)
    bin_low = np.zeros(nbins, np.int64)
    bin_high = np.zeros(nbins, np.int64)
    bin_n = np.zeros(nbins, np.int64)
    node_bin = np.zeros(N, np.int64)
    node_slot = np.zeros(N, np.int64)
    heap = [(0.0, b) for b in range(nbins)]
    heapq.heapify(heap)
    for n in nodes:
        while True:
            k, b = heapq.heappop(heap)
            cur = max(bin_low[b] / cap_l, bin_high[b] / cap_h)
            if bin_n[b] >= 128:
                continue
            if k < cur - 1e-12:         # stale key: reinsert
                heapq.heappush(heap, (cur, b))
                continue
            break
        node_bin[n] = b
        node_slot[n] = bin_n[b]
        bin_n[b] += 1
        bin_low[b] += deg_low[n]
        bin_high[b] += deg_high[n]
        if bin_n[b] < 128:
            heapq.heappush(
                heap, (max(bin_low[b] / cap_l, bin_high[b] / cap_h), b))
    return node_bin, node_slot


def _host_prep(feat, W_fc, Wq, Wk, src, dst):
    """Shard edges by dst into (core, group, src-half, parity) gather calls
    with uniform padding.  Returns T_low/T_high, the shared gather index
    array, the fp8 DoubleRow one-hot mask M1x, node permutation, c0, deg."""
    half = (src >= SPLIT).astype(np.int64)
    deg_low = np.bincount(dst[half == 0], minlength=N)
    deg_high = np.bincount(dst[half == 1], minlength=N)
    node_bin, node_slot = _pack_nodes(deg_low, deg_high)

    bin_of = node_bin[dst]
    blk_of = bin_of % NBLK
    counts_bh = np.bincount(bin_of * 2 + half, minlength=CORES * NBLK * 2)
    T_low = int(np.ceil(counts_bh[0::2].max() / 128))
    T_high = int(np.ceil(counts_bh[1::2].max() / 128))
    T_blk = T_low + T_high
    GT = GPB * T_blk
    ntiles = NG * GT

    g_of = blk_of // GPB
    par_of = blk_of % GPB
    core_of = bin_of // NBLK
    gkey = ((core_of * NG + g_of) * 2 + half) * GPB + par_of
    nkeys = CORES * NG * 2 * GPB
    counts = np.bincount(gkey, minlength=nkeys)

    order = np.argsort(gkey, kind="stable")
    gk_s = gkey[order]
    src_s = src[order]
    drel_s = node_slot[dst][order]

    starts = np.zeros(nkeys + 1, np.int64)
    np.cumsum(counts, out=starts[1:])
    pos = np.arange(E, dtype=np.int64) - starts[gk_s]

    ks = np.arange(nkeys)
    k_g = (ks // (2 * GPB)) % NG
    k_half = (ks // GPB) % 2
    k_par = ks % GPB
    k_tile_base = k_g * GT + np.where(
        k_half == 0, k_par * T_low, GPB * T_low + k_par * T_high)

    slot = k_tile_base[gk_s] * 128 + pos
    lane = slot % 128
    tl = slot // 128
    core_s = gk_s // (NG * 2 * GPB)

    # shared gather indices (16-row wrap, tiled to 128 partitions)
    ncols = ntiles * 8
    idx_val = np.where(gk_s % (2 * GPB) < GPB, src_s, src_s - SPLIT) \
        .astype(np.int16)
    idx16 = np.zeros((CORES, 16, ncols), np.int16)
    col = k_tile_base[gk_s] * 8 + pos // 16
    row = pos % 16
    idx16[core_s, row, col] = idx_val
    idx16 = np.tile(idx16, (1, 8, 1))

    # fp8 one-hot mask in DoubleRow layout: tile tl occupies partitions
    # (tl%2)*64..(tl%2)*64+64, cols (tl//2)*256 + ihalf*128 + lane, where
    # slot s = ihalf*64 + krow.  1 where dstrel == s (pad cols all-zero).
    m1 = np.zeros((CORES, 128, (ntiles // 2) * 256), FP8)
    krow = drel_s % 64
    ihalf = drel_s // 64
    m1[core_s, (tl % 2) * 64 + krow, (tl // 2) * 256 + ihalf * 128 + lane] \
        = np.float32(1.0)

    perm = np.full((CORES * NBLK, 128), -1, np.int64)
    perm[node_bin, node_slot] = np.arange(N)

    # softmax shift: any constant >= max(e) keeps exp in range
    Qh = feat @ Wq
    Kh = feat @ Wk
    emax = -np.inf
    for i in range(0, E, 131072):
        sl = slice(i, min(i + 131072, E))
        e = np.einsum("ij,ij->i", Qh[dst[sl]], Kh[src[sl]]) / SCALE
        emax = max(emax, float(e.max()))
    c0 = float(emax)

    deg = deg_low + deg_high
    return T_low, T_high, idx16, m1, perm, c0, deg


def _build_program(T_low, T_high, c0):
    T_blk = T_low + T_high
    GT = GPB * T_blk
    ntiles = NG * GT
    ncols = ntiles * 8

    nc = bacc.Bacc("TRN2", target_bir_lowering=False, debug=False,
                   num_devices=CORES)

    featT_d = nc.dram_tensor("featT", [128, NPADT], dt.bfloat16,
                             kind="ExternalInput")
    featR_d = nc.dram_tensor("featR", [NPADT, 128], dt.bfloat16,
                             kind="ExternalInput")
    featTq_d = nc.dram_tensor("featTq", [128, NBLK * 128], dt.bfloat16,
                              kind="ExternalInput")
    Wfc_d = nc.dram_tensor("Wfc", [128, 128], dt.bfloat16, kind="ExternalInput")
    B_d = nc.dram_tensor("B", [128, 128], dt.bfloat16, kind="ExternalInput")
    gidx_d = nc.dram_tensor("gidx", [128, ncols], dt.int16, kind="ExternalInput")
    M1x_d = nc.dram_tensor("M1x", [128, (ntiles // 2) * 256], dt.float8e4,
                           kind="ExternalInput")
    IBS_d = nc.dram_tensor("IBS", [128, 256], dt.float8e4, kind="ExternalInput")
    bias_d = nc.dram_tensor("bias", [128, 1], dt.float32, kind="ExternalInput")
    Htab_d = nc.dram_tensor("Htab", [NPADT, 128], dt.bfloat16)
    rst_d = nc.dram_tensor("rst", [NBLK * BLK, 128], dt.float32,
                           kind="ExternalOutput")
    Htab8 = Htab_d.ap().bitcast(dt.float32)    # [NPADT, 65] f32 view

    with tile.TileContext(nc) as tc:
        nc.gpsimd.load_library(mlp_lib)
        import contextlib
        with contextlib.ExitStack() as st:
            cp = st.enter_context(tc.tile_pool(name="const", bufs=1))
            # gidx first: the t=0 K gathers need it
            gidx_sb = cp.tile([128, ncols], dt.int16, tag="gidx")
            nc.sync.dma_start(out=gidx_sb[:], in_=gidx_d.ap())
            featTq_sb = cp.tile([128, NBLK * 128], dt.bfloat16, tag="featTq")
            nc.sync.dma_start(out=featTq_sb[:], in_=featTq_d.ap())
            B_sb = cp.tile([128, 128], dt.bfloat16, tag="bmat")
            nc.sync.dma_start(out=B_sb[:], in_=B_d.ap())
            Wfc_sb = cp.tile([128, 128], dt.bfloat16, tag="wfc")
            nc.sync.dma_start(out=Wfc_sb[:], in_=Wfc_d.ap())
            # one-hot rhs duplicated across both partition halves so odd
            # tiles (lhsT at base partition 64) have a matching-base rhs
            IBS_sb = cp.tile([128, 2, 128], dt.float8e4, tag="ibs")
            nc.sync.dma_start(out=IBS_sb[:],
                              in_=IBS_d.ap().rearrange("p (i c) -> p i c", i=2))
            bias_sb = cp.tile([128, 1], dt.float32, tag="bias")
            nc.sync.dma_start(out=bias_sb[:], in_=bias_d.ap())
            ones_sb = cp.tile([128, 1], dt.bfloat16, tag="ones")
            nc.vector.memset(ones_sb[:], 1.0)
            QT_sb = cp.tile([128, NBLK, 128], dt.bfloat16, tag="qt")

            n_oct = (GT + OCT - 1) // OCT
            nLt = GPB * T_low               # low tiles per group
            nL = nLt * 128
            nH = GPB * T_high * 128

            gkt = st.enter_context(tc.tile_pool(name="gkt", bufs=PFK + 1))
            ghb = st.enter_context(tc.tile_pool(name="ghb", bufs=3))
            m1p = st.enter_context(tc.tile_pool(name="m1p", bufs=DEFER + 3))
            m2p = st.enter_context(
                tc.tile_pool(name="m2p", bufs=(DEFER + 1) * n_oct + 1))
            epp = st.enter_context(tc.tile_pool(name="ep", bufs=4))
            ftw = st.enter_context(tc.tile_pool(name="ftw", bufs=3))
            p1 = st.enter_context(tc.tile_pool(name="p1", bufs=5))
            p1p = st.enter_context(tc.tile_pool(name="p1p", bufs=2, space="PSUM"))
            spsu = st.enter_context(tc.tile_pool(name="spsu", bufs=2, space="PSUM"))
            upsu = st.enter_context(tc.tile_pool(name="upsu", bufs=1, space="PSUM"))

            KT = [None] * NG
            M1all = [None] * NG
            M2all = [[None] * n_oct for _ in range(NG)]

            def _gatherK(g):
                """Both-halves transposed K gathers from featR (no table
                dependency — runs as soon as gidx is loaded)."""
                cb = g * GT * 8
                KT[g] = gkt.tile([128, 1, GT * 128], dt.bfloat16,
                                 tag="kt", name=f"kt_{g}")
                nc.gpsimd.dma_gather(
                    out_ap=KT[g][:, :, 0:nL],
                    in_ap=featR_d.ap()[0:SPLIT, :],
                    idxs_ap=gidx_sb[:, cb:cb + nL // 16],
                    num_idxs=nL, num_idxs_reg=nL,
                    elem_size=128, elem_step=128,
                    transpose=True, single_packet=False)
                nc.gpsimd.dma_gather(
                    out_ap=KT[g][:, :, nL:GT * 128],
                    in_ap=featR_d.ap()[SPLIT:NPADT, :],
                    idxs_ap=gidx_sb[:, cb + nL // 16:cb + GT * 8],
                    num_idxs=nH, num_idxs_reg=nH,
                    elem_size=128, elem_step=128,
                    transpose=True, single_packet=False)

            def _loadM1(g, eng):
                M1all[g] = m1p.tile([128, (GT // 2) * 256], dt.float8e4,
                                    tag="m1", name=f"m1_{g}")
                eng.dma_start(
                    out=M1all[g][:],
                    in_=M1x_d.ap()[:, g * (GT // 2) * 256:
                                   (g + 1) * (GT // 2) * 256])

            def _parity(t):
                if t < nLt:
                    return t // T_low
                return (t - nLt) // T_high

            def _sgroup(g):
                """Scores + mask + exp for group g (PE + ACT); M2 kept in
                SBUF for the deferred U pass."""
                KTg = KT[g]
                M1v = M1all[g].rearrange("p (t i c) -> p t i c", i=2, c=128)
                for o in range(n_oct):
                    t0 = o * OCT
                    on = min(OCT, GT - t0)
                    So = spsu.tile([128, OCT, 128], dt.float32, space="PSUM",
                                   tag="soct", name=f"soct_{g}_{o}")
                    for ti in range(on):
                        t = t0 + ti
                        nc.tensor.matmul(
                            So[:, ti, :],
                            lhsT=KTg[:, 0, t * 128:(t + 1) * 128],
                            rhs=QT_sb[:, GPB * g + _parity(t), :],
                            start=True, stop=False)
                        hp = (t % 2) * 64
                        if USE_DR:
                            nc.tensor.matmul(
                                So[:, ti, :],
                                lhsT=M1v[hp:hp + 64, t // 2, :, :],
                                rhs=IBS_sb[hp:hp + 64, :, :],
                                start=False, stop=True,
                                perf_mode=mybir.MatmulPerfMode.DoubleRow)
                        else:
                            for ih in range(2):
                                nc.tensor.matmul(
                                    So[:, ti, :],
                                    lhsT=M1v[hp:hp + 64, t // 2, ih, :],
                                    rhs=IBS_sb[hp:hp + 64, ih, :],
                                    start=False, stop=(ih == 1))
                    M2all[g][o] = m2p.tile([128, OCT, 128], dt.bfloat16,
                                           tag="m2oct", name=f"m2oct_{g}_{o}")
                    nc.scalar.activation(M2all[g][o][:, 0:on, :], So[:, 0:on, :],
                                         mybir.ActivationFunctionType.Exp,
                                         bias=bias_sb[:, 0:1],
                                         scale=1.0 / SCALE)

            def _ugroup(g):
                """h gathers land earlier (Pool); U/Dn accumulation (PE) and
                the per-block epilogue (DVE) + output DMA (SP)."""
                cb = g * GT * 8
                # h rows gathered into cols 0:64 (f32 view); col 64 memset
                # to f32 1.0, whose bf16 view is (0.0, 1.0) — so one matmul
                # per tile yields U in cols 0:128 and the denominator in
                # col 129 of the same accumulation group.
                HBL = ghb.tile([128, nLt, 64], dt.float32, tag="hbl",
                               name=f"hbl_{g}")
                nc.gpsimd.dma_gather(
                    out_ap=HBL[:, :, :],
                    in_ap=Htab8[0:SPLIT, :],
                    idxs_ap=gidx_sb[:, cb:cb + nL // 16],
                    num_idxs=nL, num_idxs_reg=nL,
                    elem_size=64, elem_step=64, single_packet=False)
                HBH = ghb.tile([128, GPB * T_high, 64], dt.float32, tag="hbh",
                               name=f"hbh_{g}")
                nc.gpsimd.dma_gather(
                    out_ap=HBH[:, :, :],
                    in_ap=Htab8[SPLIT:NPADT, :],
                    idxs_ap=gidx_sb[:, cb + nL // 16:cb + GT * 8],
                    num_idxs=nH, num_idxs_reg=nH,
                    elem_size=64, elem_step=64, single_packet=False)
                HBLb = HBL.bitcast(dt.bfloat16)   # [128, nLt, 128]
                HBHb = HBH.bitcast(dt.bfloat16)

                def _hb(t):
                    if t < nLt:
                        return HBLb[:, t, :]
                    return HBHb[:, t - nLt, :]

                # all four accumulators share one PSUM bank; their byte
                # regions are disjoint so the coarse zero-region check is
                # safely skipped
                UD = upsu.tile([128, 2, 136], dt.float32, space="PSUM",
                               tag="ud", name=f"ud_{g}")
                U = [UD[:, p, 0:128] for p in range(GPB)]
                Dn = [UD[:, p, 128:136] for p in range(GPB)]
                for o in range(n_oct):
                    t0 = o * OCT
                    for ti in range(min(OCT, GT - t0)):
                        t = t0 + ti
                        p = _parity(t)
                        first = (t == p * T_low)
                        last = (t == nLt + (p + 1) * T_high - 1)
                        nc.tensor.matmul(U[p],
                                         lhsT=M2all[g][o][:, ti, :],
                                         rhs=_hb(t), start=first, stop=last,
                                         skip_group_check=True)
                        nc.tensor.matmul(Dn[p][:, 0:1],
                                         lhsT=M2all[g][o][:, ti, :],
                                         rhs=ones_sb[:],
                                         start=first, stop=last,
                                         skip_group_check=True)
                for p in range(GPB):
                    b = GPB * g + p
                    dg = epp.tile([128, 1], dt.float32, tag="dg")
                    nc.vector.tensor_scalar(
                        out=dg[:], in0=Dn[p][:, 0:1], scalar1=1e-30,
                        scalar2=None, op0=mybir.AluOpType.add)
                    rr = epp.tile([128, 1], dt.float32, tag="rr")
                    nc.vector.reciprocal(rr[:], dg[:])
                    ro = epp.tile([128, 128], dt.float32, tag="ro")
                    nc.vector.tensor_scalar(
                        out=ro[:], in0=U[p], scalar1=rr[:, 0:1],
                        scalar2=None, op0=mybir.AluOpType.mult)
                    nc.sync.dma_start(
                        out=rst_d.ap()[b * BLK:(b + 1) * BLK, :],
                        in_=ro[:])

            # ---- t=0 streams: K prefetch (Pool), M1 for early groups (ACT)
            for g0 in range(PFK):
                _gatherK(g0)
            for g0 in range(DEFER + 1):
                _loadM1(g0, nc.scalar)

            # ---- QT tiles (only featTq + B needed): PE warm-up ----
            for b in range(NBLK):
                psq4 = p1p.tile([128, 4, 128], dt.float32, tag="hp")
                psq = psq4[:, 0, :]
                nc.tensor.matmul(
                    psq, lhsT=B_sb[:],
                    rhs=featTq_sb[:, 128 * b:128 * (b + 1)],
                    start=True, stop=True)
                if b % 2 == 0:
                    nc.scalar.activation(QT_sb[:, b, :], psq,
                                         mybir.ActivationFunctionType.Copy)
                else:
                    nc.vector.tensor_copy(out=QT_sb[:, b, :], in_=psq)

            # ---- Htab build interleaved with early score groups ----
            CHT = 40                       # featT window: 40 tiles per chunk
            NCH = (NT_GLOBAL + CHT - 1) // CHT
            ftw_sb = [None] * NCH

            def _loadchunk(c):
                t0c = c * CHT
                tnc = min(CHT, NT_GLOBAL - t0c)
                ftw_sb[c] = ftw.tile([128, CHT * 128], dt.bfloat16,
                                     tag="ftw", name=f"ftw_{c}")
                nc.sync.dma_start(
                    out=ftw_sb[c][:, 0:tnc * 128],
                    in_=featT_d.ap()[:, t0c * 128:(t0c + tnc) * 128])

            _loadchunk(0)
            _loadchunk(1)
            evA, evD = 17000.0, 3000.0    # debited for exp/M1 (ACT), QT (DVE)
            NPACK = (NT_GLOBAL + 3) // 4
            sgi = 0                        # next early score-group to issue
            wev = []                       # pending (ev tile, gn, row0) writes
            wi = 0
            ev = None
            for pk in range(NPACK):
                t0p = pk * 4
                qn = min(4, NT_GLOBAL - t0p)
                c = t0p // CHT
                if t0p % CHT == 0 and c + 2 < NCH:
                    _loadchunk(c + 2)
                if t0p % 8 == 0:
                    ev = p1.tile([128, 8, 128], dt.bfloat16, tag="he")
                ps = p1p.tile([128, 4, 128], dt.float32, tag="hp")
                for qi in range(qn):
                    g = t0p + qi
                    cg = g // CHT
                    nc.tensor.matmul(
                        ps[:, qi, :],
                        lhsT=ftw_sb[cg][:, (g % CHT) * 128:(g % CHT + 1) * 128],
                        rhs=Wfc_sb[:], start=True, stop=True)
                # GPSIMD cannot read PSUM (BIR verifier): ACT/DVE evacuate
                eo = (t0p % 8)
                if evA + qn * 143 <= evD + qn * 165:
                    nc.scalar.activation(ev[:, eo:eo + qn, :], ps[:, 0:qn, :],
                                         mybir.ActivationFunctionType.Copy)
                    evA += qn * 143
                else:
                    nc.vector.tensor_copy(out=ev[:, eo:eo + qn, :],
                                          in_=ps[:, 0:qn, :])
                    evD += qn * 165
                if eo + qn == 8 or t0p + qn == NT_GLOBAL:
                    r0 = (t0p - eo) * 128
                    r1 = (t0p + qn) * 128
                    out_ap = Htab_d.ap()[r0:r1, :] \
                        .rearrange("(t p) c -> p t c", p=128)
                    weng = nc.sync if wi % 3 != 2 else nc.scalar
                    wi += 1
                    weng.dma_start(out=out_ap, in_=ev[:, 0:(eo + qn), :])
                # early score groups spaced through the Htab build
                if sgi < DEFER and pk == 20 * (sgi + 1):
                    _sgroup(sgi)
                    sgi += 1

            # ---- steady loop: gathers / deferred scores / U pass ----
            for g in range(NG):
                if g + PFK < NG:
                    _gatherK(g + PFK)
                if g + DEFER + 1 < NG:
                    _loadM1(g + DEFER + 1, nc.sync)
                if g + DEFER < NG:
                    _sgroup(g + DEFER)
                _ugroup(g)
    nc.finalize()
    return nc


def _make_in_maps(feat, W_fc, Wq, Wk, idx16, m1, perm, c0):
    featT = feat.T.astype(BF16)
    featT_pad = np.zeros((128, NPADT), BF16)
    featT_pad[:, :N] = featT
    featR_pad = np.zeros((NPADT, 128), BF16)
    featR_pad[:N, :] = feat.astype(BF16)
    Bm = (Wq @ Wk.T).astype(BF16)
    Wfc_b = W_fc.astype(BF16)
    # IBS DoubleRow rhs: [64, 2, 128] flattened, 256*delta(i*64+k == d)
    IBS = np.zeros((64, 2, 128), np.float32)
    for i in range(2):
        IBS[np.arange(64), i, i * 64 + np.arange(64)] = BIGSCALE
    IBS = np.tile(IBS.reshape(64, 256), (2, 1)).astype(FP8)
    bias = np.full((128, 1), -c0 - BIGSCALE / SCALE, np.float32)

    in_maps = []
    for c in range(CORES):
        pc = perm[c * NBLK:(c + 1) * NBLK].reshape(-1)
        fq = np.zeros((128, NBLK * 128), BF16)
        valid = pc >= 0
        fq[:, valid] = featT[:, pc[valid]]
        in_maps.append({
            "featT": featT_pad,
            "featR": featR_pad,
            "featTq": fq,
            "Wfc": Wfc_b,
            "B": Bm,
            "gidx": np.ascontiguousarray(idx16[c]),
            "M1x": np.ascontiguousarray(m1[c]),
            "IBS": IBS,
            "bias": bias,
        })
    return in_maps


_CACHE = {}


def kernel(feat, loc, W_fc, Wq, Wk, Wq2, Wk2, G_w, embed, boundaries,
           src, dst, inter_ids, **_ignored):
    feat = np.asarray(feat, np.float32)
    W_fc = np.asarray(W_fc, np.float32)
    Wq = np.asarray(Wq, np.float32)
    Wk = np.asarray(Wk, np.float32)
    src = np.asarray(src).astype(np.int64)
    dst = np.asarray(dst).astype(np.int64)

    T_low, T_high, idx16, m1, perm, c0, deg = _host_prep(
        feat, W_fc, Wq, Wk, src, dst)

    key = (T_low, T_high, round(c0, 4))
    if key not in _CACHE:
        _CACHE[key] = _build_program(T_low, T_high, c0)
    nc = _CACHE[key]

    in_maps = _make_in_maps(feat, W_fc, Wq, Wk, idx16, m1, perm, c0)

    res = run_bass_kernel_spmd(nc, in_maps, core_ids=list(range(CORES)))
    out = np.zeros((N, F), np.float32)
    for c in range(CORES):
        pc = perm[c * NBLK:(c + 1) * NBLK].reshape(-1)
        valid = pc >= 0
        out[pc[valid]] = res.results[c]["rst"][valid]
    out[deg == 0] = 0.0
    return out


# revision 29
# speedup vs baseline: 1.0888x; 1.0888x over previous
"""Trainium2 Bass kernel for nn_DistAttn (GNN edge-softmax message passing).

Strategy (8 NeuronCores, SPMD single program), "design V":
  - Destination-node sharding: nodes packed into 320 bins (8 cores x 40
    blocks of <=128 dst slots) by a degree-balancing greedy; every edge
    lives on exactly one core; per-block edge counts near-uniform.
  - Score refactor: e = f_dst (Wq Wk^T) f_src^T, so K rows are the RAW
    bf16 features (host-supplied table featR, available at t=0) and the
    per-block query tiles are QT[c,d] = B^T @ feat_blk^T with the tiny
    host-computed B = Wq @ Wk^T.  Only the h table (feat @ W_fc) is
    built on device (phase 1), as bf16 DRAM rows split low/high at
    row 22528 so gather indices fit int16 and the low half completes
    early.
  - Phase 1 (per core): Htab = feat @ W_fc for all 40064 padded nodes
    (PE), PSUM evacuated by a cost-balanced mix of Pool/ACT/DVE copies,
    streamed to DRAM on SP/ACT/Pool queues.  QT tiles computed first and
    kept in SBUF.  K^T gathers for the first groups run on Pool from
    t~0 (featR needs no compute).
  - Phase 2: edges in groups of GPB=2 blocks.  Per group, Pool issues
    transposed gathers of K^T [c,j] from featR (elem 256B) and int64-
    viewed row gathers of h (32 elems/idx) from Htab, sharing one int16
    index array.  Per 128-edge tile, PE computes all-pairs scores
    S^T[j,d] = K^T.T @ QT into PSUM, then adds 256*onehot(dst) with a
    single fp8 DoubleRow matmul (lhsT = host-built one-hot in [64,2,128]
    layout, rhs = 256*I in the matching layout; 0.5 cycles/row).  One
    ACT Exp over an 8-tile PSUM region yields the masked softmax
    numerators M2[j,d] = exp(e_j - c0) * onehot (mismatches underflow:
    exp(-256/sqrt(128)) ~ 1.5e-10).  Two more PE matmuls accumulate
    U[d,:] += M2^T @ h and denom[d] += M2^T @ 1.  Block epilogue (DVE):
    reciprocal + scaled copy, then DMA the rows out on SP.
  - exp shift c0 >= max e is host-computed, so no segment-max pass.
  - The host unpermutes the output rows and zeroes deg-0 nodes.
"""

import sys

sys.path.insert(0, "/opt/trn_rl_repo")

import numpy as np

import concourse.bacc as bacc
import concourse.mybir as mybir
import concourse.tile as tile
from concourse.bass_utils import run_bass_kernel_spmd
from concourse.library_config import mlp as mlp_lib

dt = mybir.dt
BF16 = dt.np(dt.bfloat16)
FP8 = dt.np(dt.float8e4)

N = 40000
E = 640000
F = 128
CORES = 8
NPC = N // CORES            # 5000 dst nodes per core
BLK = 128                   # dst nodes per block
NBLK = (NPC + BLK - 1) // BLK   # 40 blocks per core
SPLIT = 22528               # low/high table split; both halves < 2**15 rows
NPADT = 40064               # node count padded to 128 multiple (313 tiles)
NT_GLOBAL = NPADT // 128    # 313
SCALE = float(np.sqrt(np.float32(F)))
BIGSCALE = 224.0            # exactly representable in fp8e4 (max 240)
GPB = 2                     # blocks per gather group
NG = NBLK // GPB            # groups per core
OCT = 8                     # tiles per ACT exp call
PFK = 5                     # K-gather groups prefetched from t=0
USE_DR = True               # fp8 DoubleRow mask matmul (0.5 cycles/row)
USE_I64 = False             # int64 gathers unsupported by the HW backend
DEFER = 3                   # S/exp groups run ahead of their U groups


def _pack_nodes(deg_low, deg_high):
    """Assign nodes to CORES*NBLK bins (<=128 nodes each), balancing the
    per-bin low/high edge counts to minimize gather padding."""
    import heapq
    nbins = CORES * NBLK
    nodes = np.argsort(-(deg_low + deg_high), kind="stable")
    # caps target whole tile counts: ceil(avg/128) tiles per (bin, half)
    cap_l = max(np.ceil(float(deg_low.sum()) / nbins / 128) * 128 - 2.0, 1.0)
    cap_h = max(np.ceil(float(deg_high.sum()) / nbins / 128) * 128 - 2.0, 1.0)
    bin_low = np.zeros(nbins, np.int64)
    bin_high = np.zeros(nbins, np.int64)
    bin_n = np.zeros(nbins, np.int64)
    node_bin = np.zeros(N, np.int64)
    node_slot = np.zeros(N, np.int64)
    heap = [(0.0, b) for b in range(nbins)]
    heapq.heapify(heap)
    for n in nodes:
        while True:
            k, b = heapq.heappop(heap)
            cur = max(bin_low[b] / cap_l, bin_high[b] / cap_h)
            if bin_n[b] >= 128:
                continue
            if k < cur - 1e-12:         # stale key: reinsert
                heapq.heappush(heap, (cur, b))
                continue
            break
        node_bin[n] = b
        node_slot[n] = bin_n[b]
        bin_n[b] += 1
        bin_low[b] += deg_low[n]
        bin_high[b] += deg_high[n]
        if bin_n[b] < 128:
            heapq.heappush(
                heap, (max(bin_low[b] / cap_l, bin_high[b] / cap_h), b))
    return node_bin, node_slot


def _host_prep(feat, W_fc, Wq, Wk, src, dst):
    """Shard edges by dst into (core, group, src-half, parity) gather calls
    with uniform padding.  Returns T_low/T_high, the shared gather index
    array, the fp8 DoubleRow one-hot mask M1x, node permutation, c0, deg."""
    half = (src >= SPLIT).astype(np.int64)
    deg_low = np.bincount(dst[half == 0], minlength=N)
    deg_high = np.bincount(dst[half == 1], minlength=N)
    node_bin, node_slot = _pack_nodes(deg_low, deg_high)

    bin_of = node_bin[dst]
    blk_of = bin_of % NBLK
    counts_bh = np.bincount(bin_of * 2 + half, minlength=CORES * NBLK * 2)
    T_low = int(np.ceil(counts_bh[0::2].max() / 128))
    T_high = int(np.ceil(counts_bh[1::2].max() / 128))
    T_blk = T_low + T_high
    GT = GPB * T_blk
    ntiles = NG * GT

    g_of = blk_of // GPB
    par_of = blk_of % GPB
    core_of = bin_of // NBLK
    gkey = ((core_of * NG + g_of) * 2 + half) * GPB + par_of
    nkeys = CORES * NG * 2 * GPB
    counts = np.bincount(gkey, minlength=nkeys)

    order = np.argsort(gkey, kind="stable")
    gk_s = gkey[order]
    src_s = src[order]
    drel_s = node_slot[dst][order]

    starts = np.zeros(nkeys + 1, np.int64)
    np.cumsum(counts, out=starts[1:])
    pos = np.arange(E, dtype=np.int64) - starts[gk_s]

    ks = np.arange(nkeys)
    k_g = (ks // (2 * GPB)) % NG
    k_half = (ks // GPB) % 2
    k_par = ks % GPB
    k_tile_base = k_g * GT + np.where(
        k_half == 0, k_par * T_low, GPB * T_low + k_par * T_high)

    slot = k_tile_base[gk_s] * 128 + pos
    lane = slot % 128
    tl = slot // 128
    core_s = gk_s // (NG * 2 * GPB)

    # shared gather indices (16-row wrap, tiled to 128 partitions).
    # Values are "node-block" indices: node n of local tile t (=n//128),
    # partition p lives at block ((t//2)*128+p)*2 + t%2, so that tile
    # PAIRS of one partition are contiguous 512B runs in the tables
    # (full-width DMA writes) while gather elems stay 256B.
    ncols = ntiles * 8
    rel = np.where(gk_s % (2 * GPB) < GPB, src_s, src_s - SPLIT)
    rt, rp = rel // 128, rel % 128
    idx_val = (((rt // 2) * 128 + rp) * 2 + rt % 2).astype(np.int16)
    idx16 = np.zeros((CORES, 16, ncols), np.int16)
    col = k_tile_base[gk_s] * 8 + pos // 16
    row = pos % 16
    idx16[core_s, row, col] = idx_val
    idx16 = np.tile(idx16, (1, 8, 1))

    # fp8 one-hot mask in DoubleRow layout: tile tl occupies partitions
    # (tl%2)*64..(tl%2)*64+64, cols (tl//2)*256 + ihalf*128 + lane, where
    # slot s = ihalf*64 + krow.  1 where dstrel == s (pad cols all-zero).
    m1 = np.zeros((CORES, 128, (ntiles // 2) * 256), FP8)
    krow = drel_s % 64
    ihalf = drel_s // 64
    m1[core_s, (tl % 2) * 64 + krow, (tl // 2) * 256 + ihalf * 128 + lane] \
        = np.float32(1.0)

    perm = np.full((CORES * NBLK, 128), -1, np.int64)
    perm[node_bin, node_slot] = np.arange(N)

    # softmax shift: any constant >= max(e) keeps exp in range
    Qh = feat @ Wq
    Kh = feat @ Wk
    emax = -np.inf
    for i in range(0, E, 131072):
        sl = slice(i, min(i + 131072, E))
        e = np.einsum("ij,ij->i", Qh[dst[sl]], Kh[src[sl]]) / SCALE
        emax = max(emax, float(e.max()))
    c0 = float(emax)

    deg = deg_low + deg_high
    return T_low, T_high, idx16, m1, perm, c0, deg


def _build_program(T_low, T_high, c0):
    T_blk = T_low + T_high
    GT = GPB * T_blk
    ntiles = NG * GT
    ncols = ntiles * 8

    nc = bacc.Bacc("TRN2", target_bir_lowering=False, debug=False,
                   num_devices=CORES)

    featT_d = nc.dram_tensor("featT", [128, NPADT], dt.bfloat16,
                             kind="ExternalInput")
    featR_d = nc.dram_tensor("featR", [22528 + 17664, 128], dt.bfloat16,
                             kind="ExternalInput")
    featTq_d = nc.dram_tensor("featTq", [128, NBLK * 128], dt.bfloat16,
                              kind="ExternalInput")
    Wfc_d = nc.dram_tensor("Wfc", [128, 128], dt.bfloat16, kind="ExternalInput")
    B_d = nc.dram_tensor("B", [128, 128], dt.bfloat16, kind="ExternalInput")
    gidx_d = nc.dram_tensor("gidx", [128, ncols], dt.int16, kind="ExternalInput")
    M1x_d = nc.dram_tensor("M1x", [128, (ntiles // 2) * 256], dt.float8e4,
                           kind="ExternalInput")
    IBS_d = nc.dram_tensor("IBS", [128, 256], dt.float8e4, kind="ExternalInput")
    bias_d = nc.dram_tensor("bias", [128, 1], dt.float32, kind="ExternalInput")
    Htab_d = nc.dram_tensor("Htab", [20096, 256], dt.bfloat16)
    rst_d = nc.dram_tensor("rst", [NBLK * BLK, 128], dt.float32,
                           kind="ExternalOutput")
    # f32 view reshaped to one 256B node-block per row
    Htab8 = Htab_d.ap().bitcast(dt.float32).rearrange(
        "r (u e) -> (r u) e", u=2)                 # [40192, 64]

    with tile.TileContext(nc) as tc:
        nc.gpsimd.load_library(mlp_lib)
        import contextlib
        with contextlib.ExitStack() as st:
            cp = st.enter_context(tc.tile_pool(name="const", bufs=1))
            # gidx first: the t=0 K gathers need it
            gidx_sb = cp.tile([128, ncols], dt.int16, tag="gidx")
            nc.sync.dma_start(out=gidx_sb[:], in_=gidx_d.ap())
            featTq_sb = cp.tile([128, NBLK * 128], dt.bfloat16, tag="featTq")
            nc.sync.dma_start(out=featTq_sb[:], in_=featTq_d.ap())
            B_sb = cp.tile([128, 128], dt.bfloat16, tag="bmat")
            nc.sync.dma_start(out=B_sb[:], in_=B_d.ap())
            Wfc_sb = cp.tile([128, 128], dt.bfloat16, tag="wfc")
            nc.sync.dma_start(out=Wfc_sb[:], in_=Wfc_d.ap())
            # one-hot rhs duplicated across both partition halves so odd
            # tiles (lhsT at base partition 64) have a matching-base rhs
            IBS_sb = cp.tile([128, 2, 128], dt.float8e4, tag="ibs")
            nc.sync.dma_start(out=IBS_sb[:],
                              in_=IBS_d.ap().rearrange("p (i c) -> p i c", i=2))
            bias_sb = cp.tile([128, 1], dt.float32, tag="bias")
            nc.sync.dma_start(out=bias_sb[:], in_=bias_d.ap())
            ones_sb = cp.tile([128, 1], dt.bfloat16, tag="ones")
            nc.vector.memset(ones_sb[:], 1.0)
            # zero the unwritten pair-half tail of Htab (tile 312 has no
            # pair partner) so gather views see no poison
            zro_sb = cp.tile([128, 128], dt.bfloat16, tag="zro")
            nc.vector.memset(zro_sb[:], 0.0)
            nc.scalar.dma_start(out=Htab_d.ap()[19968:20096, 128:256],
                                in_=zro_sb[:])
            QT_sb = cp.tile([128, NBLK, 128], dt.bfloat16, tag="qt")

            n_oct = (GT + OCT - 1) // OCT
            nLt = GPB * T_low               # low tiles per group
            nL = nLt * 128
            nH = GPB * T_high * 128

            gkt = st.enter_context(tc.tile_pool(name="gkt", bufs=PFK + 1))
            ghb = st.enter_context(tc.tile_pool(name="ghb", bufs=3))
            m1p = st.enter_context(tc.tile_pool(name="m1p", bufs=DEFER + 3))
            m2p = st.enter_context(
                tc.tile_pool(name="m2p", bufs=(DEFER + 1) * n_oct + 1))
            epp = st.enter_context(tc.tile_pool(name="ep", bufs=4))
            ftw = st.enter_context(tc.tile_pool(name="ftw", bufs=3))
            p1 = st.enter_context(tc.tile_pool(name="p1", bufs=5))
            p1p = st.enter_context(tc.tile_pool(name="p1p", bufs=2, space="PSUM"))
            spsu = st.enter_context(tc.tile_pool(name="spsu", bufs=2, space="PSUM"))
            upsu = st.enter_context(tc.tile_pool(name="upsu", bufs=1, space="PSUM"))

            KT = [None] * NG
            M1all = [None] * NG
            M2all = [[None] * n_oct for _ in range(NG)]

            def _gatherK(g):
                """Both-halves transposed K gathers from featR (no table
                dependency — runs as soon as gidx is loaded)."""
                cb = g * GT * 8
                KT[g] = gkt.tile([128, 1, GT * 128], dt.bfloat16,
                                 tag="kt", name=f"kt_{g}")
                nc.gpsimd.dma_gather(
                    out_ap=KT[g][:, :, 0:nL],
                    in_ap=featR_d.ap()[0:22528, :],
                    idxs_ap=gidx_sb[:, cb:cb + nL // 16],
                    num_idxs=nL, num_idxs_reg=nL,
                    elem_size=128, elem_step=128,
                    transpose=True, single_packet=False)
                nc.gpsimd.dma_gather(
                    out_ap=KT[g][:, :, nL:GT * 128],
                    in_ap=featR_d.ap()[22528:22528 + 17664, :],
                    idxs_ap=gidx_sb[:, cb + nL // 16:cb + GT * 8],
                    num_idxs=nH, num_idxs_reg=nH,
                    elem_size=128, elem_step=128,
                    transpose=True, single_packet=False)

            def _loadM1(g, eng):
                M1all[g] = m1p.tile([128, (GT // 2) * 256], dt.float8e4,
                                    tag="m1", name=f"m1_{g}")
                eng.dma_start(
                    out=M1all[g][:],
                    in_=M1x_d.ap()[:, g * (GT // 2) * 256:
                                   (g + 1) * (GT // 2) * 256])

            def _parity(t):
                if t < nLt:
                    return t // T_low
                return (t - nLt) // T_high

            def _sgroup(g):
                """Scores + mask + exp for group g (PE + ACT); M2 kept in
                SBUF for the deferred U pass."""
                KTg = KT[g]
                M1v = M1all[g].rearrange("p (t i c) -> p t i c", i=2, c=128)
                for o in range(n_oct):
                    t0 = o * OCT
                    on = min(OCT, GT - t0)
                    So = spsu.tile([128, OCT, 128], dt.float32, space="PSUM",
                                   tag="soct", name=f"soct_{g}_{o}")
                    for ti in range(on):
                        t = t0 + ti
                        nc.tensor.matmul(
                            So[:, ti, :],
                            lhsT=KTg[:, 0, t * 128:(t + 1) * 128],
                            rhs=QT_sb[:, GPB * g + _parity(t), :],
                            start=True, stop=False)
                        hp = (t % 2) * 64
                        if USE_DR:
                            nc.tensor.matmul(
                                So[:, ti, :],
                                lhsT=M1v[hp:hp + 64, t // 2, :, :],
                                rhs=IBS_sb[hp:hp + 64, :, :],
                                start=False, stop=True,
                                perf_mode=mybir.MatmulPerfMode.DoubleRow)
                        else:
                            for ih in range(2):
                                nc.tensor.matmul(
                                    So[:, ti, :],
                                    lhsT=M1v[hp:hp + 64, t // 2, ih, :],
                                    rhs=IBS_sb[hp:hp + 64, ih, :],
                                    start=False, stop=(ih == 1))
                    M2all[g][o] = m2p.tile([128, OCT, 128], dt.bfloat16,
                                           tag="m2oct", name=f"m2oct_{g}_{o}")
                    nc.scalar.activation(M2all[g][o][:, 0:on, :], So[:, 0:on, :],
                                         mybir.ActivationFunctionType.Exp,
                                         bias=bias_sb[:, 0:1],
                                         scale=1.0 / SCALE)

            def _ugroup(g):
                """h gathers land earlier (Pool); U/Dn accumulation (PE) and
                the per-block epilogue (DVE) + output DMA (SP)."""
                cb = g * GT * 8
                # h rows gathered into cols 0:64 (f32 view); col 64 memset
                # to f32 1.0, whose bf16 view is (0.0, 1.0) — so one matmul
                # per tile yields U in cols 0:128 and the denominator in
                # col 129 of the same accumulation group.
                HBL = ghb.tile([128, nLt, 64], dt.float32, tag="hbl",
                               name=f"hbl_{g}")
                nc.gpsimd.dma_gather(
                    out_ap=HBL[:, :, :],
                    in_ap=Htab8[0:22528, :],
                    idxs_ap=gidx_sb[:, cb:cb + nL // 16],
                    num_idxs=nL, num_idxs_reg=nL,
                    elem_size=64, elem_step=64, single_packet=False)
                HBH = ghb.tile([128, GPB * T_high, 64], dt.float32, tag="hbh",
                               name=f"hbh_{g}")
                nc.gpsimd.dma_gather(
                    out_ap=HBH[:, :, :],
                    in_ap=Htab8[22528:40192, :],
                    idxs_ap=gidx_sb[:, cb + nL // 16:cb + GT * 8],
                    num_idxs=nH, num_idxs_reg=nH,
                    elem_size=64, elem_step=64, single_packet=False)
                HBLb = HBL.bitcast(dt.bfloat16)   # [128, nLt, 128]
                HBHb = HBH.bitcast(dt.bfloat16)

                def _hb(t):
                    if t < nLt:
                        return HBLb[:, t, :]
                    return HBHb[:, t - nLt, :]

                # all four accumulators share one PSUM bank; their byte
                # regions are disjoint so the coarse zero-region check is
                # safely skipped
                UD = upsu.tile([128, 2, 136], dt.float32, space="PSUM",
                               tag="ud", name=f"ud_{g}")
                U = [UD[:, p, 0:128] for p in range(GPB)]
                Dn = [UD[:, p, 128:136] for p in range(GPB)]
                for o in range(n_oct):
                    t0 = o * OCT
                    for ti in range(min(OCT, GT - t0)):
                        t = t0 + ti
                        p = _parity(t)
                        first = (t == p * T_low)
                        last = (t == nLt + (p + 1) * T_high - 1)
                        nc.tensor.matmul(U[p],
                                         lhsT=M2all[g][o][:, ti, :],
                                         rhs=_hb(t), start=first, stop=last,
                                         skip_group_check=True)
                        nc.tensor.matmul(Dn[p][:, 0:1],
                                         lhsT=M2all[g][o][:, ti, :],
                                         rhs=ones_sb[:],
                                         start=first, stop=last,
                                         skip_group_check=True)
                for p in range(GPB):
                    b = GPB * g + p
                    dg = epp.tile([128, 1], dt.float32, tag="dg")
                    nc.vector.tensor_scalar(
                        out=dg[:], in0=Dn[p][:, 0:1], scalar1=1e-30,
                        scalar2=None, op0=mybir.AluOpType.add)
                    rr = epp.tile([128, 1], dt.float32, tag="rr")
                    nc.vector.reciprocal(rr[:], dg[:])
                    ro = epp.tile([128, 128], dt.float32, tag="ro")
                    nc.vector.tensor_scalar(
                        out=ro[:], in0=U[p], scalar1=rr[:, 0:1],
                        scalar2=None, op0=mybir.AluOpType.mult)
                    nc.sync.dma_start(
                        out=rst_d.ap()[b * BLK:(b + 1) * BLK, :],
                        in_=ro[:])

            # ---- t=0 streams: K prefetch (Pool), M1 for early groups (ACT)
            for g0 in range(PFK):
                _gatherK(g0)
            for g0 in range(DEFER + 1):
                _loadM1(g0, nc.scalar)

            # ---- QT tiles (only featTq + B needed): PE warm-up ----
            for b in range(NBLK):
                psq4 = p1p.tile([128, 4, 128], dt.float32, tag="hp")
                psq = psq4[:, 0, :]
                nc.tensor.matmul(
                    psq, lhsT=B_sb[:],
                    rhs=featTq_sb[:, 128 * b:128 * (b + 1)],
                    start=True, stop=True)
                nc.vector.tensor_copy(out=QT_sb[:, b, :], in_=psq)

            # ---- Htab build interleaved with early score groups ----
            CHT = 40                       # featT window: 40 tiles per chunk
            NCH = (NT_GLOBAL + CHT - 1) // CHT
            ftw_sb = [None] * NCH

            def _loadchunk(c):
                t0c = c * CHT
                tnc = min(CHT, NT_GLOBAL - t0c)
                ftw_sb[c] = ftw.tile([128, CHT * 128], dt.bfloat16,
                                     tag="ftw", name=f"ftw_{c}")
                nc.sync.dma_start(
                    out=ftw_sb[c][:, 0:tnc * 128],
                    in_=featT_d.ap()[:, t0c * 128:(t0c + tnc) * 128])

            _loadchunk(0)
            _loadchunk(1)
            evA, evD = 8000.0, 6600.0     # debited for M1 (ACT), QT (DVE)
            NPACK = (NT_GLOBAL + 3) // 4
            sgi = 0                        # next early score-group to issue
            wev = []                       # pending (ev tile, gn, row0) writes
            wi = 0
            ev = None
            for pk in range(NPACK):
                t0p = pk * 4
                qn = min(4, NT_GLOBAL - t0p)
                c = t0p // CHT
                if t0p % CHT == 0 and c + 2 < NCH:
                    _loadchunk(c + 2)
                if t0p % 8 == 0:
                    ev = p1.tile([128, 8, 128], dt.bfloat16, tag="he")
                ps = p1p.tile([128, 4, 128], dt.float32, tag="hp")
                for qi in range(qn):
                    g = t0p + qi
                    cg = g // CHT
                    nc.tensor.matmul(
                        ps[:, qi, :],
                        lhsT=ftw_sb[cg][:, (g % CHT) * 128:(g % CHT + 1) * 128],
                        rhs=Wfc_sb[:], start=True, stop=True)
                # GPSIMD cannot read PSUM (BIR verifier): ACT/DVE evacuate
                eo = (t0p % 8)
                if evA + qn * 143 <= evD + qn * 165:
                    nc.scalar.activation(ev[:, eo:eo + qn, :], ps[:, 0:qn, :],
                                         mybir.ActivationFunctionType.Copy)
                    evA += qn * 143
                else:
                    nc.vector.tensor_copy(out=ev[:, eo:eo + qn, :],
                                          in_=ps[:, 0:qn, :])
                    evD += qn * 165
                if eo + qn == 8 or t0p + qn == NT_GLOBAL:
                    tw0 = t0p - eo
                    twn = eo + qn
                    hb, tl = (0, tw0) if tw0 < 176 else (11264, tw0 - 176)
                    v0 = tl // 2
                    if twn % 2 == 0:
                        sl = Htab_d.ap()[hb + v0 * 128:
                                         hb + (v0 + twn // 2) * 128, :]
                        out_ap = sl.rearrange("(v p) (w c) -> p v w c",
                                              p=128, w=2)
                        weng = nc.sync if wi % 3 != 2 else nc.scalar
                        wi += 1
                        weng.dma_start(
                            out=out_ap,
                            in_=ev[:, 0:twn, :].rearrange(
                                "p (v w) c -> p v w c", w=2))
                        continue
                    else:
                        sl = Htab_d.ap()[hb + v0 * 128:hb + (v0 + 1) * 128,
                                         0:128]
                        out_ap = sl.rearrange("(v p) c -> p v c", p=128)
                        assert twn == 1
                    weng = nc.sync if wi % 3 != 2 else nc.scalar
                    wi += 1
                    weng.dma_start(out=out_ap, in_=ev[:, 0:twn, :])
                # early score groups spaced through the Htab build
                if sgi < DEFER and pk == 20 * (sgi + 1):
                    _sgroup(sgi)
                    sgi += 1

            # ---- steady loop: gathers / deferred scores / U pass ----
            for g in range(NG):
                if g + PFK < NG:
                    _gatherK(g + PFK)
                if g + DEFER + 1 < NG:
                    _loadM1(g + DEFER + 1, nc.sync)
                if g + DEFER < NG:
                    _sgroup(g + DEFER)
                _ugroup(g)
    nc.finalize()
    return nc


def _make_in_maps(feat, W_fc, Wq, Wk, idx16, m1, perm, c0):
    featT = feat.T.astype(BF16)
    featT_pad = np.zeros((128, NPADT), BF16)
    featT_pad[:, :N] = featT
    # featR rows in node-block order (matching gather indices): low
    # nodes at block b, high nodes at 22528 + b_high
    featR_pad = np.zeros((22528 + 17664, 128), BF16)
    n = np.arange(N)
    t_l = n // 128
    b = ((t_l // 2) * 128 + n % 128) * 2 + t_l % 2
    lo = n < SPLIT
    featR_pad[b[lo], :] = feat[lo].astype(BF16)
    nh = n[~lo] - SPLIT
    t_h = nh // 128
    bh = ((t_h // 2) * 128 + nh % 128) * 2 + t_h % 2
    featR_pad[22528 + bh, :] = feat[~lo].astype(BF16)
    Bm = (Wq @ Wk.T).astype(BF16)
    Wfc_b = W_fc.astype(BF16)
    # IBS DoubleRow rhs: [64, 2, 128] flattened, 256*delta(i*64+k == d)
    IBS = np.zeros((64, 2, 128), np.float32)
    for i in range(2):
        IBS[np.arange(64), i, i * 64 + np.arange(64)] = BIGSCALE
    IBS = np.tile(IBS.reshape(64, 256), (2, 1)).astype(FP8)
    bias = np.full((128, 1), -c0 - BIGSCALE / SCALE, np.float32)

    in_maps = []
    for c in range(CORES):
        pc = perm[c * NBLK:(c + 1) * NBLK].reshape(-1)
        fq = np.zeros((128, NBLK * 128), BF16)
        valid = pc >= 0
        fq[:, valid] = featT[:, pc[valid]]
        in_maps.append({
            "featT": featT_pad,
            "featR": featR_pad,
            "featTq": fq,
            "Wfc": Wfc_b,
            "B": Bm,
            "gidx": np.ascontiguousarray(idx16[c]),
            "M1x": np.ascontiguousarray(m1[c]),
            "IBS": IBS,
            "bias": bias,
        })
    return in_maps


_CACHE = {}


def kernel(feat, loc, W_fc, Wq, Wk, Wq2, Wk2, G_w, embed, boundaries,
           src, dst, inter_ids, **_ignored):
    feat = np.asarray(feat, np.float32)
    W_fc = np.asarray(W_fc, np.float32)
    Wq = np.asarray(Wq, np.float32)
    Wk = np.asarray(Wk, np.float32)
    src = np.asarray(src).astype(np.int64)
    dst = np.asarray(dst).astype(np.int64)

    T_low, T_high, idx16, m1, perm, c0, deg = _host_prep(
        feat, W_fc, Wq, Wk, src, dst)

    key = (T_low, T_high, round(c0, 4))
    if key not in _CACHE:
        _CACHE[key] = _build_program(T_low, T_high, c0)
    nc = _CACHE[key]

    in_maps = _make_in_maps(feat, W_fc, Wq, Wk, idx16, m1, perm, c0)

    res = run_bass_kernel_spmd(nc, in_maps, core_ids=list(range(CORES)))
    out = np.zeros((N, F), np.float32)
    for c in range(CORES):
        pc = perm[c * NBLK:(c + 1) * NBLK].reshape(-1)
        valid = pc >= 0
        out[pc[valid]] = res.results[c]["rst"][valid]
    out[deg == 0] = 0.0
    return out
